# revision 42
# baseline (speedup 1.0000x reference)
"""Trainium2 Bass kernel for nn_EquiMLP (gnn_message_passing).

Reference computation per batch b (B=16, n_cgs=128, n_atoms=W=1024, knn=8,
E = n_cgs*knn = 1024 edges):
  d2     = pairwise sq dists of cg_xyz[b]           [128,128]
  knbrs  = argsort(d2)[:, 1:9]                      [128,8]
  dv     = nbr_xyz - cg_xyz  (flattened)            [E,3]
  dist   = |dv|                                     [E]
  h0 = relu(dist*w0 + b0); h = relu(h0@w1 + b1)     [E,W]
  coeffs = h@w2 + b2                                [E,W]
  dx     = coeffs^T @ dv                            [W,3]
  cg     = an^T @ dx        (an = assign_norm[b])   [128,3]
  out    = cg_xyz[idx] - cg[idx] + dx               [1024,3]

Key algebraic restructure: coeffs (E x n_atoms, 67MB across batches) is never
materialized.  Since dx = (h@w2 + b2)^T @ dv = w2^T @ (h^T@dv) + b2 (x) sum(dv),
we contract over edges FIRST: g = h^T @ dv  [W,3], then dx = w2^T@g + b2 (x) s.
This removes one of the two E*W*W matmuls entirely.

Sharding: data-parallel over batch; core c computes batches {2c, 2c+1} fused
(2048 edges).  knn/argsort preprocessing and final index-gather assembly run
on host (tiny); all matmuls/activations run on device in bf16 with fp32 psum
accumulation.

Device layout notes (TRN2 matmul: out[M,N] = lhsT[K,M].T @ rhs[K,N], K =
partition dim):
  h0T  [j1-tile 128, e 2048] generated on ScalarE: relu(dist_bc*w0 + b0)
  z    [e 128, j2 1024] psum  = sum_k h0T_k.T @ w1_k   (+ ones^T@b1 rank-1)
  h    [e 128, j2 1024] bf16 sbuf = relu(z)  (VectorE, psum->sbuf cast)
  gT_b [3, j 1024] psum += dv_tile.T @ h_tile          (contract over edges)
  g2   [j 128-tiles, 6] via PE transpose of gT
  dx   [o 128-tiles, 6] psum = sum_k w2_k.T @ g2_k + b2 (x) s2
  cg   [c 128, 6] psum = sum_i an_i.T @ dx_i

Hardware constraint honored throughout: a PE instruction (Matmult/Ldweights)
can carry at most ONE semaphore wait, so "observer" matmuls absorb DMA-queue
waits into psum columns that are subsequently overwritten (start=True), and
the gT matmuls are software-pipelined one chunk behind the z matmuls so psum
recycling needs no extra DVE wait on the z matmuls.
"""

import numpy as np
import ml_dtypes

B, N_CGS, N_ATOMS, KNN = 16, 128, 1024, 8
W = N_ATOMS
N_CORES = 8
BPC = B // N_CORES          # batches per core = 2
E = N_CGS * KNN             # edges per batch = 1024
E2 = BPC * E                # edges per core = 2048
ET = E2 // 128              # e-tiles per core = 16
KT = W // 128               # k-tiles = 8
NB = BPC * 3                # fused xyz columns = 6

_BF = ml_dtypes.bfloat16
_BUILT = {}
LAST_RESULT = None          # BassKernelResults of the last run (for test.py)
TRACE = False               # set True from test.py to profile


def _build():
    import concourse.bass as bass
    import concourse.mybir as mybir
    from concourse.tile import TileContext
    from concourse.vector_clock import ScopedClock

    # The kernel-tail drain gets one wait per live semaphore (~11), but every
    # TPB instruction has a single HW wait slot.  Pre-emit standalone sync
    # wait_ge instructions (one per sem) before the drain; the post-pass then
    # strips the drain down to its last wait.
    _orig_dab = TileContext._drain_and_barrier

    def _patched_dab(self, tick_clock, wait_clock):
        probe = self.nc.sync.nop(hint="drain_wait_probe").ins
        wait_clock.add_sem_waits(probe, ScopedClock({None: tick_clock.global_clock}))
        waits = list(probe.sync_info.on_wait) if probe.sync_info and probe.sync_info.on_wait else []
        if waits:
            probe.sync_info.on_wait = [waits[-1]]
        handles = {h.name: h for h in self.sems.allocated().values()}
        for w in waits:
            h = handles.get(str(w.ant_name))
            if h is not None:
                self.nc.sync.wait_ge(h, w.wait_value)
            else:
                raise RuntimeError(f"no sem handle for {w.ant_name}")
        _orig_dab(self, tick_clock, wait_clock)

    TileContext._drain_and_barrier = _patched_dab

    bf = mybir.dt.bfloat16
    f32 = mybir.dt.float32
    nc = bass.Bass()

    # ---- DRAM I/O ----
    P = lambda name, shape, dt_: nc.declare_dram_parameter(name, shape, dt_, isOutput=False)
    dist_bc_d = P("dist_bc", [128, E2], bf)          # dist broadcast over partitions
    w0c_d = P("w0c", [128, KT], f32)                 # per-partition scales
    b0c_d = P("b0c", [128, KT], f32)
    w1t_d = P("w1t", [128, KT * W], bf)              # k-tiled w1 (rows k*128..)
    w2t_d = P("w2t", [128, KT * W], bf)              # k-tiled w2
    b1r_d = P("b1r", [1, W], bf)
    b2r_d = P("b2r", [1, W], bf)
    ones_d = P("ones", [1, 128], bf)
    s2_d = P("s2", [1, NB], bf)                      # per-batch sum(dv)
    dvt_d = P("dvt", [128, ET * 3], bf)              # e-tiled dist_vec
    ant_d = P("ant", [128, BPC * KT * 128], bf)      # i-tiled assign_norm
    id3_d = P("id3", [3, 3], f32)
    dxo_d = nc.declare_dram_parameter("dxo", [128, KT * NB], f32, isOutput=True)
    cgo_d = nc.declare_dram_parameter("cgo", [128, NB], f32, isOutput=True)
    scr1_d = nc.dram_tensor("scr1", [1, 8], f32)
    scr2_d = nc.dram_tensor("scr2", [1, 8], f32)

    with TileContext(nc) as tc:
        from contextlib import ExitStack
        with ExitStack() as ctx:
            csb = ctx.enter_context(tc.tile_pool(name="consts", bufs=1))
            wsb = ctx.enter_context(tc.tile_pool(name="weights", bufs=1))
            hsb = ctx.enter_context(tc.tile_pool(name="h", bufs=3))
            osb = ctx.enter_context(tc.tile_pool(name="outs", bufs=1))
            ps_g = ctx.enter_context(tc.tile_pool(name="ps_g", bufs=1, space="PSUM"))

            # ---- SBUF tiles + loads ----
            dist_bc = csb.tile([128, E2], bf)
            w0c = csb.tile([128, KT], f32)
            b0c = csb.tile([128, KT], f32)
            b1r = csb.tile([1, W], bf)
            b2r = csb.tile([1, W], bf)
            ones = csb.tile([1, 128], bf)
            s2 = csb.tile([1, NB], bf)
            dvt = csb.tile([128, ET * 3], bf)
            ant = csb.tile([128, BPC * KT * 128], bf)
            id3 = csb.tile([3, 3], f32)
            w1t = wsb.tile([128, KT * W], bf)
            w2t = wsb.tile([128, KT * W], bf)
            h0T = wsb.tile([128, KT * E2], bf)

            sd = nc.sync.dma_start
            # Startup-critical DMAs go on the GpSimd queue (its sequencer
            # exits the preamble earliest) with warmup inputs first, so the
            # HAM warmup matmuls can start as soon as PE's preamble ends.
            gd = nc.gpsimd.dma_start
            gd(out=ones[:], in_=ones_d[:])
            gd(out=b1r[:], in_=b1r_d[:])
            gd(out=w0c[:], in_=w0c_d[:])
            gd(out=b0c[:], in_=b0c_d[:])
            gd(out=dist_bc[:, 0:E2 // 2], in_=dist_bc_d[:, 0:E2 // 2])
            gd(out=dist_bc[:, E2 // 2:], in_=dist_bc_d[:, E2 // 2:])
            sd(out=b2r[:], in_=b2r_d[:])
            sd(out=s2[:], in_=s2_d[:])
            sd(out=id3[:], in_=id3_d[:])
            sd(out=dvt[:], in_=dvt_d[:])
            for k in range(KT):
                sd(out=w1t[:, k * W:(k + 1) * W], in_=w1t_d[:, k * W:(k + 1) * W])
            sd(out=ant[:], in_=ant_d[:])
            for k in range(KT):
                sd(out=w2t[:, k * W:(k + 1) * W], in_=w2t_d[:, k * W:(k + 1) * W])

            # gT accumulators live across the whole main loop.  Packed as
            # [35, 512] (row 0-2 = j 0:512, rows 32-34 = j 512:1024) so each
            # fits ONE psum bank, freeing a bank for the HAM warmup tile.
            gT = [ps_g.tile([35, 512], f32, tag=f"gT{b}", name=f"gT{b}")
                  for b in range(BPC)]

            Relu = mybir.ActivationFunctionType.Relu

            with tc.tile_pool(name="ps_z", bufs=2, space="PSUM") as ps_z:
                # ---- observers: absorb DMA-queue waits into PE, one per inst.
                # They write z-psum columns that k==0/start=True later clears.
                zobs = ps_z.tile([128, 1024], f32, tag="z")
                # HAM warmup FIRST in the PE stream (only needs ones/b1r,
                # which are DMA'd early): dense real matmuls into a dedicated
                # psum bank nothing else touches, keeping the PE activity
                # monitor hot through the phase-A/DMA startup window.
                wup = ps_g.tile([128, 512], f32, tag="wup", name="wup")
                NWUP = 16
                for i in range(NWUP):
                    nc.tensor.matmul(wup[:, :], ones[:, :], b1r[:, 0:512],
                                     start=(i == 0), stop=(i == NWUP - 1))
                obs_targets = [s2, dvt, id3] + \
                    [w1t[:, k * W: k * W + 128] for k in range(KT)]
                # out [1,1] = t[:, :1].T @ t[:, :1] — absorbs one DMA wait each
                for t in obs_targets:
                    nc.tensor.matmul(zobs[0:1, 0:1], t[:, 0:1], t[:, 0:1],
                                     start=True, stop=True)

                # ---- ACT observers: absorb phase-A input DMA waits, one per op
                sscr = csb.tile([1, 8], f32, name="sscr")
                act_obs = [nc.scalar.copy(sscr[0:1, i:i + 1], t)
                           for i, t in enumerate(
                               (dist_bc[0:1, 0:1],
                                dist_bc[0:1, E2 // 2:E2 // 2 + 1],
                                w0c[0:1, 0:1], b0c[0:1, 0:1]))]

                # ---- phase A: h0T generation, e-superchunk-major so the z loop
                # can start after the first superchunk.
                sc_edges = [0, 256, 512, 1024, 1536, 2048]
                from concourse.bass import _add_dep_helper
                for sc in range(len(sc_edges) - 1):
                    lo, hi = sc_edges[sc], sc_edges[sc + 1]
                    for m in range(KT):
                        a = nc.scalar.activation(
                            h0T[:, m * E2 + lo: m * E2 + hi],
                            dist_bc[:, lo:hi],
                            Relu,
                            bias=b0c[:, m:m + 1],
                            scale=w0c[:, m:m + 1],
                        )
                        if sc == 0:
                            # force the observers to schedule before phase A
                            for o in act_obs:
                                _add_dep_helper(a.ins, o.ins, sync=False,
                                                reason="act-obs order")

                # ---- main loop: z -> relu -> (pipelined) gT
                h_tiles = [None] * ET

                gT_last_by_ec = {}

                def emit_z(ec):
                    z = ps_z.tile([128, 1024], f32, tag="z", name="z")
                    for n in range(2):
                        # rank-1 bias row opens the accumulation region
                        # (start=True, no h0T dep, absorbs psum epoch waits)
                        nc.tensor.matmul(
                            z[:, n * 512:(n + 1) * 512],
                            ones[:, :],
                            b1r[:, n * 512:(n + 1) * 512],
                            start=True, stop=False)
                        for k in range(KT):
                            nc.tensor.matmul(
                                z[:, n * 512:(n + 1) * 512],
                                h0T[:, k * E2 + ec * 128: k * E2 + (ec + 1) * 128],
                                w1t[:, k * W + n * 512: k * W + (n + 1) * 512],
                                start=False, stop=(k == KT - 1))
                    h = hsb.tile([128, W], bf, tag="h")
                    nc.vector.tensor_scalar_max(h[:], z[:], 0.0)
                    h_tiles[ec] = h

                last_gT = [None]

                def emit_gT(ec):
                    b = ec // KT
                    lec = ec % KT
                    for n in range(2):
                        last_gT[0] = nc.tensor.matmul(
                            gT[b][32 * n:32 * n + 3, :],
                            dvt[:, ec * 3:(ec + 1) * 3],
                            h_tiles[ec][:, n * 512:(n + 1) * 512],
                            start=(lec == 0), stop=(lec == KT - 1))
                    gT_last_by_ec[ec] = last_gT[0]

                for ec in range(ET):
                    emit_z(ec)
                    if ec >= 1:
                        emit_gT(ec - 1)
                emit_gT(ET - 1)

            # ---- tail ----
            with tc.tile_pool(name="ps_t", bufs=1, space="PSUM") as ps_t:
                gT_sb = [osb.tile([3, W], f32, tag=f"gTs{b}", name=f"gTs{b}")
                         for b in range(BPC)]
                for b in range(BPC):
                    nc.scalar.copy(gT_sb[b][:, 0:512], gT[b][0:3, :])
                    nc.scalar.copy(gT_sb[b][:, 512:1024], gT[b][32:35, :])

                # transpose gT -> g2 [128, KT*6], per batch (partition base 0)
                g2p = ps_t.tile([128, KT * NB], f32, tag="g2p")
                # PE observer for the ps_t pool-overlap waits; forced after
                # the final gT matmul so its DVE released-zone component is
                # already observed by the PE clock (single wait left).
                g2p_obs = nc.tensor.matmul(g2p[0:1, 0:1], id3[:, 0:1],
                                           id3[:, 0:1], start=True, stop=True)
                _add_dep_helper(g2p_obs.ins, last_gT[0].ins, sync=False,
                                reason="tail-obs after last gT")
                for t in range(KT):
                    for b in range(BPC):
                        nc.tensor.transpose(
                            g2p[:, t * NB + 3 * b: t * NB + 3 * b + 3],
                            gT_sb[b][:, t * 128:(t + 1) * 128],
                            id3[:, :])
                g2 = osb.tile([128, KT * NB], bf)
                vscr = csb.tile([1, 8], f32, name="vscr")
                # DVE observer: absorb the PE (transposes-done) wait.  The g2
                # copy runs only after ALL transposes: a slice-wise pipeline
                # here races PE-writes vs DVE-reads on the single g2p psum
                # bank (hard fault / corruption).
                nc.vector.tensor_copy(vscr[0:1, 0:1], g2p[0:1, 0:1])
                nc.vector.tensor_copy(g2[:], g2p[:])

                # dx = sum_k w2_k.T @ g2_k + b2 (x) s2
                dxp = ps_t.tile([128, KT * NB], f32, tag="dxp")
                # observers for w2t chunks (+ g2 DVE) land in dxp col 0
                for k in range(KT):
                    o = nc.tensor.matmul(dxp[0:1, 0:1],
                                         w2t[:, k * W: k * W + 1],
                                         w2t[:, k * W: k * W + 1],
                                         start=True, stop=True)
                    if k == 0:
                        _add_dep_helper(o.ins, last_gT[0].ins, sync=False,
                                        reason="tail-obs after last gT")
                nc.tensor.matmul(dxp[0:1, 0:1], g2[:, 0:1], g2[:, 0:1],
                                 start=True, stop=True)
                for oc in range(KT):
                    nc.tensor.matmul(
                        dxp[:, oc * NB:(oc + 1) * NB],
                        b2r[:, oc * 128:(oc + 1) * 128],
                        s2[:, :],
                        start=True, stop=False)
                    for k in range(KT):
                        nc.tensor.matmul(
                            dxp[:, oc * NB:(oc + 1) * NB],
                            w2t[:, k * W + oc * 128: k * W + (oc + 1) * 128],
                            g2[:, k * NB:(k + 1) * NB],
                            start=False, stop=(k == KT - 1))

                dxo = osb.tile([128, KT * NB], f32)
                dxb = osb.tile([128, KT * NB], bf)
                # observer: DVE absorbs the PE (dx-done) wait first
                nc.vector.tensor_copy(vscr[0:1, 1:2], dxp[0:1, 0:1])
                nc.vector.tensor_copy(dxo[:], dxp[:])
                nc.vector.tensor_copy(dxb[:], dxp[:])
                # dummy 4B DMA absorbs the DVE wait; the real out-DMA then
                # carries only its DRAM-page WAR wait (1 slot each).
                nc.sync.dma_start(out=scr1_d[0:1, 0:1], in_=dxo[0:1, 0:1])
                nc.sync.dma_start(out=dxo_d[:], in_=dxo[:])

                # cg = sum_i an_i.T @ dx_i   per batch
                cgp = ps_t.tile([128, NB], f32, tag="cgp")
                o = nc.tensor.matmul(cgp[0:1, 0:1], ant[:, 0:1], ant[:, 0:1],
                                     start=True, stop=True)
                _add_dep_helper(o.ins, last_gT[0].ins, sync=False,
                                reason="tail-obs after last gT")
                nc.tensor.matmul(cgp[0:1, 0:1], dxb[:, 0:1], dxb[:, 0:1],
                                 start=True, stop=True)
                for b in range(BPC):
                    for it in range(KT):
                        nc.tensor.matmul(
                            cgp[:, 3 * b:3 * (b + 1)],
                            ant[:, (b * KT + it) * 128:(b * KT + it + 1) * 128],
                            dxb[:, it * NB + 3 * b: it * NB + 3 * b + 3],
                            start=(it == 0), stop=(it == KT - 1))
                cgo = osb.tile([128, NB], f32)
                nc.vector.tensor_copy(cgo[:], cgp[:])
                nc.sync.dma_start(out=scr2_d[0:1, 0:1], in_=cgo[0:1, 0:1])
                nc.sync.dma_start(out=cgo_d[:], in_=cgo[:])

    TileContext._drain_and_barrier = _orig_dab
    _strip_self_waits(nc)
    return nc


# Engine instruction families -> the engine's own completion-sem prefix.
# An instruction waiting on its OWN engine's sem is trivially satisfied at
# runtime for the strictly in-order ACT/DVE queues (RAR/epoch bookkeeping the
# scheduler fails to elide), but it consumes the single HW wait slot.  PE
# self-waits are NOT stripped (psum fill/drain overlap makes them real).
_SELF_SEM = {
    "InstTensorScalarPtr": "DVE_",
    "InstTensorCopy": "DVE_",
    "InstTensorTensor": "DVE_",
    "InstMemset": "DVE_",
    "InstActivation": "Activation_",
}


def _strip_self_waits(nc):
    for bb in nc.m.functions[0].blocks:
        for inst in bb.instructions:
            si = getattr(inst, "sync_info", None)
            if not si or not si.on_wait or len(si.on_wait) < 2:
                continue
            tn = type(inst).__name__
            if tn == "InstDrain":
                # covered by the pre-emitted wait_ge chain (_patched_dab)
                si.on_wait = [si.on_wait[-1]]
                continue
            if tn == "InstDMACopy":
                # A DMA waiting on the sem of its OWN hardware queue is
                # redundant: per-queue descriptor execution is FIFO.
                own = {str(u.ant_name) for u in (si.on_update or [])}
                kept = [w for w in si.on_wait if str(w.ant_name) not in own]
                if len(kept) != len(si.on_wait) and kept:
                    si.on_wait = kept
                continue
            pref = _SELF_SEM.get(tn)
            if pref is None:
                continue
            kept = [w for w in si.on_wait if not str(w.ant_name).startswith(pref)]
            if len(kept) != len(si.on_wait) and kept:
                si.on_wait = kept


def _enable_ldw_opt():  # unused: walrus rejects pre-split InstLdweights
    # The toolchain invokes walrus with --enable-ldw-opt=false, which keeps
    # every MATMUL serialized behind its LDWEIGHTS (~380ns instead of ~215ns
    # per N=512 bf16 matmul).  Enabling it fails on Tile's pre-split
    # InstLdweights ("not compatible with LDW optimization"), and the split
    # happens in compiled tile_legalize.rs — not reachable from here.
    import concourse.bass_utils as bu
    if getattr(bu.run_command, "_ldw_patched", False):
        return
    orig = bu.run_command

    def patched(cmd, **kw):
        cmd = [("--enable-ldw-opt=true" if c == "--enable-ldw-opt=false" else c)
               for c in cmd]
        return orig(cmd, **kw)

    patched._ldw_patched = True
    bu.run_command = patched


def _get_nc():
    if "nc" not in _BUILT:
        _BUILT["nc"] = _build()
    return _BUILT["nc"]


def _host_prep(cg_xyz):
    """Exact replication of the reference knn/edge construction (fp32)."""
    diff = cg_xyz[:, :, None, :] - cg_xyz[:, None, :, :]
    d2 = (diff ** 2).sum(-1)                      # [B, 128, 128] fp32
    knbrs = np.argsort(d2, axis=-1, kind="stable")[:, :, 1:KNN + 1]
    nbr = np.stack([cg_xyz[b][knbrs[b]] for b in range(cg_xyz.shape[0])])
    dv = (nbr - cg_xyz[:, :, None, :]).reshape(cg_xyz.shape[0], E, 3)
    dist = np.sqrt((dv ** 2).sum(-1, keepdims=True))  # [B, E, 1]
    return dv.astype(np.float32), dist[..., 0].astype(np.float32)


def kernel(soft_assign, xyz, cg_xyz, assign_norm, assign_idx,
           w0, b0, w1, b1, w2, b2):
    global LAST_RESULT
    # If BASS_TRACE is set in an environment whose antenv lacks axon_hooks,
    # concourse's trace path would crash on import; register a stub registry
    # so tracing degrades gracefully instead.
    try:
        import antenv.axon_hooks  # noqa: F401
    except ImportError:
        import sys, types
        _m = types.ModuleType("antenv.axon_hooks")
        _m._hook = None
        _m.set_axon_ntff_profile_hook = lambda h: setattr(_m, "_hook", h)
        _m.get_axon_ntff_profile_hook = lambda: _m._hook
        sys.modules["antenv.axon_hooks"] = _m
    from concourse.bass_utils import run_bass_kernel_spmd

    soft_assign = np.asarray(soft_assign)
    xyz = np.asarray(xyz)
    cg_xyz = np.asarray(cg_xyz, dtype=np.float32)
    assign_norm = np.asarray(assign_norm, dtype=np.float32)
    idx = np.asarray(assign_idx).astype(np.int64)
    w0 = np.asarray(w0, dtype=np.float32); b0 = np.asarray(b0, dtype=np.float32)
    w1 = np.asarray(w1, dtype=np.float32); b1 = np.asarray(b1, dtype=np.float32)
    w2 = np.asarray(w2, dtype=np.float32); b2 = np.asarray(b2, dtype=np.float32)

    dv, dist = _host_prep(cg_xyz)                 # [B,E,3], [B,E]

    # shared (weight) inputs
    w0c = np.ascontiguousarray(w0.reshape(KT, 128).T, dtype=np.float32)
    b0c = np.ascontiguousarray(b0.reshape(KT, 128).T, dtype=np.float32)
    w1t = np.ascontiguousarray(
        w1.reshape(KT, 128, W).transpose(1, 0, 2).reshape(128, KT * W)).astype(_BF)
    w2t = np.ascontiguousarray(
        w2.reshape(KT, 128, W).transpose(1, 0, 2).reshape(128, KT * W)).astype(_BF)
    b1r = b1.reshape(1, W).astype(_BF)
    b2r = b2.reshape(1, W).astype(_BF)
    ones = np.ones((1, 128), dtype=_BF)
    id3 = np.eye(3, dtype=np.float32)

    in_maps = []
    for c in range(N_CORES):
        bs = slice(BPC * c, BPC * (c + 1))
        dvc = dv[bs].reshape(E2, 3)               # [2048, 3]
        distc = dist[bs].reshape(E2)
        dist_bf = distc.astype(_BF)
        dist_bc = np.ascontiguousarray(np.broadcast_to(dist_bf[None, :], (128, E2)))
        dvt = np.ascontiguousarray(
            dvc.reshape(ET, 128, 3).transpose(1, 0, 2).reshape(128, ET * 3)).astype(_BF)
        s2 = dv[bs].sum(axis=1).reshape(1, NB).astype(_BF)   # [1, 6]
        anp = assign_norm[bs]                     # [2, 1024, 128]
        ant = np.ascontiguousarray(
            anp.reshape(BPC, KT, 128, N_CGS).transpose(2, 0, 1, 3)
            .reshape(128, BPC * KT * 128)).astype(_BF)
        in_maps.append({
            "dist_bc": dist_bc, "w0c": w0c, "b0c": b0c,
            "w1t": w1t, "w2t": w2t, "b1r": b1r, "b2r": b2r,
            "ones": ones, "s2": s2, "dvt": dvt, "ant": ant, "id3": id3,
        })

    nc = _get_nc()
    res = run_bass_kernel_spmd(nc, in_maps, list(range(N_CORES)), trace=TRACE)
    LAST_RESULT = res

    xyz_recon = np.empty((B, N_ATOMS, 3), dtype=np.float32)
    for c in range(N_CORES):
        dxo = res.results[c]["dxo"]               # [128, KT*6]
        cgo = res.results[c]["cgo"]               # [128, 6]
        dx = dxo.reshape(128, KT, BPC, 3).transpose(2, 1, 0, 3).reshape(BPC, W, 3)
        cg = cgo.reshape(128, BPC, 3).transpose(1, 0, 2)      # [2, 128, 3]
        for j in range(BPC):
            b = BPC * c + j
            xyz_recon[b] = cg_xyz[b][idx] - cg[j][idx] + dx[j]

    return (soft_assign, xyz, xyz_recon)


# revision 43
# speedup vs baseline: 1.0418x; 1.0418x over previous
"""Trainium2 Bass kernel for nn_EquiMLP (gnn_message_passing).

Reference computation per batch b (B=16, n_cgs=128, n_atoms=W=1024, knn=8,
E = n_cgs*knn = 1024 edges):
  d2     = pairwise sq dists of cg_xyz[b]           [128,128]
  knbrs  = argsort(d2)[:, 1:9]                      [128,8]
  dv     = nbr_xyz - cg_xyz  (flattened)            [E,3]
  dist   = |dv|                                     [E]
  h0 = relu(dist*w0 + b0); h = relu(h0@w1 + b1)     [E,W]
  coeffs = h@w2 + b2                                [E,W]
  dx     = coeffs^T @ dv                            [W,3]
  cg     = an^T @ dx        (an = assign_norm[b])   [128,3]
  out    = cg_xyz[idx] - cg[idx] + dx               [1024,3]

Key algebraic restructure: coeffs (E x n_atoms, 67MB across batches) is never
materialized.  Since dx = (h@w2 + b2)^T @ dv = w2^T @ (h^T@dv) + b2 (x) sum(dv),
we contract over edges FIRST: g = h^T @ dv  [W,3], then dx = w2^T@g + b2 (x) s.
This removes one of the two E*W*W matmuls entirely.

Sharding: data-parallel over batch; core c computes batches {2c, 2c+1} fused
(2048 edges).  knn/argsort preprocessing and final index-gather assembly run
on host (tiny); all matmuls/activations run on device in bf16 with fp32 psum
accumulation.

Device layout notes (TRN2 matmul: out[M,N] = lhsT[K,M].T @ rhs[K,N], K =
partition dim):
  h0T  [j1-tile 128, e 2048] generated on ScalarE: relu(dist_bc*w0 + b0)
  z    [e 128, j2 1024] psum  = sum_k h0T_k.T @ w1_k   (+ ones^T@b1 rank-1)
  h    [e 128, j2 1024] bf16 sbuf = relu(z)  (VectorE, psum->sbuf cast)
  gT_b [3, j 1024] psum += dv_tile.T @ h_tile          (contract over edges)
  g2   [j 128-tiles, 6] via PE transpose of gT
  dx   [o 128-tiles, 6] psum = sum_k w2_k.T @ g2_k + b2 (x) s2
  cg   [c 128, 6] psum = sum_i an_i.T @ dx_i

Hardware constraint honored throughout: a PE instruction (Matmult/Ldweights)
can carry at most ONE semaphore wait, so "observer" matmuls absorb DMA-queue
waits into psum columns that are subsequently overwritten (start=True), and
the gT matmuls are software-pipelined one chunk behind the z matmuls so psum
recycling needs no extra DVE wait on the z matmuls.
"""

import numpy as np
import ml_dtypes

B, N_CGS, N_ATOMS, KNN = 16, 128, 1024, 8
W = N_ATOMS
N_CORES = 8
BPC = B // N_CORES          # batches per core = 2
E = N_CGS * KNN             # edges per batch = 1024
E2 = BPC * E                # edges per core = 2048
ET = E2 // 128              # e-tiles per core = 16
KT = W // 128               # k-tiles = 8
NB = BPC * 3                # fused xyz columns = 6

_BF = ml_dtypes.bfloat16
_BUILT = {}
LAST_RESULT = None          # BassKernelResults of the last run (for test.py)
TRACE = False               # set True from test.py to profile


def _build():
    import concourse.bass as bass
    import concourse.mybir as mybir
    from concourse.tile import TileContext
    from concourse.vector_clock import ScopedClock

    # The kernel-tail drain gets one wait per live semaphore (~11), but every
    # TPB instruction has a single HW wait slot.  Pre-emit standalone sync
    # wait_ge instructions (one per sem) before the drain; the post-pass then
    # strips the drain down to its last wait.
    _orig_dab = TileContext._drain_and_barrier

    def _patched_dab(self, tick_clock, wait_clock):
        probe = self.nc.sync.nop(hint="drain_wait_probe").ins
        wait_clock.add_sem_waits(probe, ScopedClock({None: tick_clock.global_clock}))
        waits = list(probe.sync_info.on_wait) if probe.sync_info and probe.sync_info.on_wait else []
        if waits:
            probe.sync_info.on_wait = [waits[-1]]
        handles = {h.name: h for h in self.sems.allocated().values()}
        for w in waits:
            h = handles.get(str(w.ant_name))
            if h is not None:
                self.nc.sync.wait_ge(h, w.wait_value)
            else:
                raise RuntimeError(f"no sem handle for {w.ant_name}")
        _orig_dab(self, tick_clock, wait_clock)

    TileContext._drain_and_barrier = _patched_dab

    bf = mybir.dt.bfloat16
    f32 = mybir.dt.float32
    nc = bass.Bass()

    # ---- DRAM I/O ----
    P = lambda name, shape, dt_: nc.declare_dram_parameter(name, shape, dt_, isOutput=False)
    dist_bc_d = P("dist_bc", [128, E2], bf)          # dist broadcast over partitions
    w0c_d = P("w0c", [128, KT], f32)                 # per-partition scales
    b0c_d = P("b0c", [128, KT], f32)
    w1t_d = P("w1t", [128, KT * W], bf)              # k-tiled w1 (rows k*128..)
    w2t_d = P("w2t", [128, KT * W], bf)              # k-tiled w2
    b1r_d = P("b1r", [1, W], bf)
    b2r_d = P("b2r", [1, W], bf)
    ones_d = P("ones", [1, 128], bf)
    s2_d = P("s2", [1, NB], bf)                      # per-batch sum(dv)
    dvt_d = P("dvt", [128, ET * 3], bf)              # e-tiled dist_vec
    ant_d = P("ant", [128, BPC * KT * 128], bf)      # i-tiled assign_norm
    id3_d = P("id3", [3, 3], f32)
    dxo_d = nc.declare_dram_parameter("dxo", [128, KT * NB], f32, isOutput=True)
    cgo_d = nc.declare_dram_parameter("cgo", [128, NB], f32, isOutput=True)
    scr1_d = nc.dram_tensor("scr1", [1, 8], f32)
    scr2_d = nc.dram_tensor("scr2", [1, 8], f32)

    with TileContext(nc) as tc:
        from contextlib import ExitStack
        with ExitStack() as ctx:
            csb = ctx.enter_context(tc.tile_pool(name="consts", bufs=1))
            wsb = ctx.enter_context(tc.tile_pool(name="weights", bufs=1))
            hsb = ctx.enter_context(tc.tile_pool(name="h", bufs=3))
            osb = ctx.enter_context(tc.tile_pool(name="outs", bufs=1))
            ps_g = ctx.enter_context(tc.tile_pool(name="ps_g", bufs=1, space="PSUM"))

            # ---- SBUF tiles + loads ----
            dist_bc = csb.tile([128, E2], bf)
            w0c = csb.tile([128, KT], f32)
            b0c = csb.tile([128, KT], f32)
            b1r = csb.tile([1, W], bf)
            b2r = csb.tile([1, W], bf)
            ones = csb.tile([1, 128], bf)
            s2 = csb.tile([1, NB], bf)
            dvt = csb.tile([128, ET * 3], bf)
            ant = csb.tile([128, BPC * KT * 128], bf)
            id3 = csb.tile([3, 3], f32)
            w1t = wsb.tile([128, KT * W], bf)
            w2t = wsb.tile([128, KT * W], bf)
            h0T = wsb.tile([128, KT * E2], bf)

            sd = nc.sync.dma_start
            # Startup-critical DMAs go on the GpSimd queue (its sequencer
            # exits the preamble earliest) with warmup inputs first, so the
            # HAM warmup matmuls can start as soon as PE's preamble ends.
            gd = nc.gpsimd.dma_start
            gd(out=ones[:], in_=ones_d[:])
            gd(out=b1r[:], in_=b1r_d[:])
            gd(out=w0c[:], in_=w0c_d[:])
            gd(out=b0c[:], in_=b0c_d[:])
            gd(out=dist_bc[:, 0:E2 // 2], in_=dist_bc_d[:, 0:E2 // 2])
            gd(out=dist_bc[:, E2 // 2:], in_=dist_bc_d[:, E2 // 2:])
            sd(out=b2r[:], in_=b2r_d[:])
            sd(out=s2[:], in_=s2_d[:])
            sd(out=id3[:], in_=id3_d[:])
            sd(out=dvt[:], in_=dvt_d[:])
            for k in range(KT):
                sd(out=w1t[:, k * W:(k + 1) * W], in_=w1t_d[:, k * W:(k + 1) * W])
            sd(out=ant[:], in_=ant_d[:])
            for k in range(KT):
                sd(out=w2t[:, k * W:(k + 1) * W], in_=w2t_d[:, k * W:(k + 1) * W])

            # gT accumulators live across the whole main loop.  Packed as
            # [35, 512] (row 0-2 = j 0:512, rows 32-34 = j 512:1024) so each
            # fits ONE psum bank, freeing a bank for the HAM warmup tile.
            gT = [ps_g.tile([35, 512], f32, tag=f"gT{b}", name=f"gT{b}")
                  for b in range(BPC)]

            Relu = mybir.ActivationFunctionType.Relu

            with tc.tile_pool(name="ps_z", bufs=2, space="PSUM") as ps_z:
                # ---- observers: absorb DMA-queue waits into PE, one per inst.
                # They write z-psum columns that k==0/start=True later clears.
                zobs = ps_z.tile([128, 1024], f32, tag="z")
                # HAM warmup FIRST in the PE stream (only needs ones/b1r,
                # which are DMA'd early): dense real matmuls into a dedicated
                # psum bank nothing else touches, keeping the PE activity
                # monitor hot through the phase-A/DMA startup window.
                wup = ps_g.tile([128, 512], f32, tag="wup", name="wup")
                NWUP = 12
                for i in range(NWUP):
                    nc.tensor.matmul(wup[:, :], ones[:, :], b1r[:, 0:512],
                                     start=(i == 0), stop=(i == NWUP - 1))
                obs_targets = [s2, dvt, id3] + \
                    [w1t[:, k * W: k * W + 128] for k in range(KT)]
                # out [1,1] = t[:, :1].T @ t[:, :1] — absorbs one DMA wait each
                for t in obs_targets:
                    nc.tensor.matmul(zobs[0:1, 0:1], t[:, 0:1], t[:, 0:1],
                                     start=True, stop=True)

                # ---- ACT observers: absorb phase-A input DMA waits, one per op
                sscr = csb.tile([1, 8], f32, name="sscr")
                act_obs = [nc.scalar.copy(sscr[0:1, i:i + 1], t)
                           for i, t in enumerate(
                               (dist_bc[0:1, 0:1],
                                dist_bc[0:1, E2 // 2:E2 // 2 + 1],
                                w0c[0:1, 0:1], b0c[0:1, 0:1]))]

                # ---- phase A: h0T generation, e-superchunk-major so the z loop
                # can start after the first superchunk.
                sc_edges = [0, 256, 512, 1024, 1536, 2048]
                from concourse.bass import _add_dep_helper
                for sc in range(len(sc_edges) - 1):
                    lo, hi = sc_edges[sc], sc_edges[sc + 1]
                    for m in range(KT):
                        a = nc.scalar.activation(
                            h0T[:, m * E2 + lo: m * E2 + hi],
                            dist_bc[:, lo:hi],
                            Relu,
                            bias=b0c[:, m:m + 1],
                            scale=w0c[:, m:m + 1],
                        )
                        if sc == 0:
                            # force the observers to schedule before phase A
                            for o in act_obs:
                                _add_dep_helper(a.ins, o.ins, sync=False,
                                                reason="act-obs order")

                # ---- main loop: z -> relu -> (pipelined) gT
                h_tiles = [None] * ET

                gT_last_by_ec = {}

                def emit_z(ec):
                    z = ps_z.tile([128, 1024], f32, tag="z", name="z")
                    for n in range(2):
                        # rank-1 bias row opens the accumulation region
                        # (start=True, no h0T dep, absorbs psum epoch waits)
                        nc.tensor.matmul(
                            z[:, n * 512:(n + 1) * 512],
                            ones[:, :],
                            b1r[:, n * 512:(n + 1) * 512],
                            start=True, stop=False)
                        for k in range(KT):
                            nc.tensor.matmul(
                                z[:, n * 512:(n + 1) * 512],
                                h0T[:, k * E2 + ec * 128: k * E2 + (ec + 1) * 128],
                                w1t[:, k * W + n * 512: k * W + (n + 1) * 512],
                                start=False, stop=(k == KT - 1))
                    h = hsb.tile([128, W], bf, tag="h")
                    nc.vector.tensor_scalar_max(h[:], z[:], 0.0)
                    h_tiles[ec] = h

                last_gT = [None]

                def emit_gT(ec):
                    b = ec // KT
                    lec = ec % KT
                    for n in range(2):
                        last_gT[0] = nc.tensor.matmul(
                            gT[b][32 * n:32 * n + 3, :],
                            dvt[:, ec * 3:(ec + 1) * 3],
                            h_tiles[ec][:, n * 512:(n + 1) * 512],
                            start=(lec == 0), stop=(lec == KT - 1))
                    gT_last_by_ec[ec] = last_gT[0]

                for ec in range(ET):
                    emit_z(ec)
                    if ec >= 1:
                        emit_gT(ec - 1)
                emit_gT(ET - 1)

            # ---- tail ----
            with tc.tile_pool(name="ps_t", bufs=1, space="PSUM") as ps_t:
                gT_sb = [osb.tile([3, W], f32, tag=f"gTs{b}", name=f"gTs{b}")
                         for b in range(BPC)]
                for b in range(BPC):
                    nc.scalar.copy(gT_sb[b][:, 0:512], gT[b][0:3, :])
                    nc.scalar.copy(gT_sb[b][:, 512:1024], gT[b][32:35, :])

                # transpose gT -> g2 [128, KT*6], per batch (partition base 0)
                g2p = ps_t.tile([128, KT * NB], f32, tag="g2p")
                # PE observer for the ps_t pool-overlap waits; forced after
                # the final gT matmul so its DVE released-zone component is
                # already observed by the PE clock (single wait left).
                g2p_obs = nc.tensor.matmul(g2p[0:1, 0:1], id3[:, 0:1],
                                           id3[:, 0:1], start=True, stop=True)
                _add_dep_helper(g2p_obs.ins, last_gT[0].ins, sync=False,
                                reason="tail-obs after last gT")
                for t in range(KT):
                    for b in range(BPC):
                        nc.tensor.transpose(
                            g2p[:, t * NB + 3 * b: t * NB + 3 * b + 3],
                            gT_sb[b][:, t * 128:(t + 1) * 128],
                            id3[:, :])
                g2 = osb.tile([128, KT * NB], bf)
                vscr = csb.tile([1, 8], f32, name="vscr")
                # DVE observer: absorb the PE (transposes-done) wait.  The g2
                # copy runs only after ALL transposes: a slice-wise pipeline
                # here races PE-writes vs DVE-reads on the single g2p psum
                # bank (hard fault / corruption).
                nc.vector.tensor_copy(vscr[0:1, 0:1], g2p[0:1, 0:1])
                nc.vector.tensor_copy(g2[:], g2p[:])

                # dx = sum_k w2_k.T @ g2_k + b2 (x) s2
                dxp = ps_t.tile([128, KT * NB], f32, tag="dxp")
                # observers for w2t chunks (+ g2 DVE) land in dxp col 0
                for k in range(KT):
                    o = nc.tensor.matmul(dxp[0:1, 0:1],
                                         w2t[:, k * W: k * W + 1],
                                         w2t[:, k * W: k * W + 1],
                                         start=True, stop=True)
                    if k == 0:
                        _add_dep_helper(o.ins, last_gT[0].ins, sync=False,
                                        reason="tail-obs after last gT")
                nc.tensor.matmul(dxp[0:1, 0:1], g2[:, 0:1], g2[:, 0:1],
                                 start=True, stop=True)
                for oc in range(KT):
                    nc.tensor.matmul(
                        dxp[:, oc * NB:(oc + 1) * NB],
                        b2r[:, oc * 128:(oc + 1) * 128],
                        s2[:, :],
                        start=True, stop=False)
                    for k in range(KT):
                        nc.tensor.matmul(
                            dxp[:, oc * NB:(oc + 1) * NB],
                            w2t[:, k * W + oc * 128: k * W + (oc + 1) * 128],
                            g2[:, k * NB:(k + 1) * NB],
                            start=False, stop=(k == KT - 1))

                dxo = osb.tile([128, KT * NB], f32)
                dxb = osb.tile([128, KT * NB], bf)
                # observer: DVE absorbs the PE (dx-done) wait first
                nc.vector.tensor_copy(vscr[0:1, 1:2], dxp[0:1, 0:1])
                nc.vector.tensor_copy(dxo[:], dxp[:])
                nc.vector.tensor_copy(dxb[:], dxp[:])
                # dummy 4B DMA absorbs the DVE wait; the real out-DMA then
                # carries only its DRAM-page WAR wait (1 slot each).
                nc.sync.dma_start(out=scr1_d[0:1, 0:1], in_=dxo[0:1, 0:1])
                nc.sync.dma_start(out=dxo_d[:], in_=dxo[:])

                # cg = sum_i an_i.T @ dx_i   per batch
                cgp = ps_t.tile([128, NB], f32, tag="cgp")
                o = nc.tensor.matmul(cgp[0:1, 0:1], ant[:, 0:1], ant[:, 0:1],
                                     start=True, stop=True)
                _add_dep_helper(o.ins, last_gT[0].ins, sync=False,
                                reason="tail-obs after last gT")
                nc.tensor.matmul(cgp[0:1, 0:1], dxb[:, 0:1], dxb[:, 0:1],
                                 start=True, stop=True)
                for b in range(BPC):
                    for it in range(KT):
                        nc.tensor.matmul(
                            cgp[:, 3 * b:3 * (b + 1)],
                            ant[:, (b * KT + it) * 128:(b * KT + it + 1) * 128],
                            dxb[:, it * NB + 3 * b: it * NB + 3 * b + 3],
                            start=(it == 0), stop=(it == KT - 1))
                cgo = osb.tile([128, NB], f32)
                nc.vector.tensor_copy(cgo[:], cgp[:])
                nc.sync.dma_start(out=scr2_d[0:1, 0:1], in_=cgo[0:1, 0:1])
                nc.sync.dma_start(out=cgo_d[:], in_=cgo[:])

    TileContext._drain_and_barrier = _orig_dab
    _strip_self_waits(nc)
    return nc


# Engine instruction families -> the engine's own completion-sem prefix.
# An instruction waiting on its OWN engine's sem is trivially satisfied at
# runtime for the strictly in-order ACT/DVE queues (RAR/epoch bookkeeping the
# scheduler fails to elide), but it consumes the single HW wait slot.  PE
# self-waits are NOT stripped (psum fill/drain overlap makes them real).
_SELF_SEM = {
    "InstTensorScalarPtr": "DVE_",
    "InstTensorCopy": "DVE_",
    "InstTensorTensor": "DVE_",
    "InstMemset": "DVE_",
    "InstActivation": "Activation_",
}


def _strip_self_waits(nc):
    for bb in nc.m.functions[0].blocks:
        for inst in bb.instructions:
            si = getattr(inst, "sync_info", None)
            if not si or not si.on_wait or len(si.on_wait) < 2:
                continue
            tn = type(inst).__name__
            if tn == "InstDrain":
                # covered by the pre-emitted wait_ge chain (_patched_dab)
                si.on_wait = [si.on_wait[-1]]
                continue
            if tn == "InstDMACopy":
                # A DMA waiting on the sem of its OWN hardware queue is
                # redundant: per-queue descriptor execution is FIFO.
                own = {str(u.ant_name) for u in (si.on_update or [])}
                kept = [w for w in si.on_wait if str(w.ant_name) not in own]
                if len(kept) != len(si.on_wait) and kept:
                    si.on_wait = kept
                continue
            pref = _SELF_SEM.get(tn)
            if pref is None:
                continue
            kept = [w for w in si.on_wait if not str(w.ant_name).startswith(pref)]
            if len(kept) != len(si.on_wait) and kept:
                si.on_wait = kept


def _enable_ldw_opt():  # unused: walrus rejects pre-split InstLdweights
    # The toolchain invokes walrus with --enable-ldw-opt=false, which keeps
    # every MATMUL serialized behind its LDWEIGHTS (~380ns instead of ~215ns
    # per N=512 bf16 matmul).  Enabling it fails on Tile's pre-split
    # InstLdweights ("not compatible with LDW optimization"), and the split
    # happens in compiled tile_legalize.rs — not reachable from here.
    import concourse.bass_utils as bu
    if getattr(bu.run_command, "_ldw_patched", False):
        return
    orig = bu.run_command

    def patched(cmd, **kw):
        cmd = [("--enable-ldw-opt=true" if c == "--enable-ldw-opt=false" else c)
               for c in cmd]
        return orig(cmd, **kw)

    patched._ldw_patched = True
    bu.run_command = patched


def _get_nc():
    if "nc" not in _BUILT:
        _BUILT["nc"] = _build()
    return _BUILT["nc"]


def _host_prep(cg_xyz):
    """Exact replication of the reference knn/edge construction (fp32)."""
    diff = cg_xyz[:, :, None, :] - cg_xyz[:, None, :, :]
    d2 = (diff ** 2).sum(-1)                      # [B, 128, 128] fp32
    knbrs = np.argsort(d2, axis=-1, kind="stable")[:, :, 1:KNN + 1]
    nbr = np.stack([cg_xyz[b][knbrs[b]] for b in range(cg_xyz.shape[0])])
    dv = (nbr - cg_xyz[:, :, None, :]).reshape(cg_xyz.shape[0], E, 3)
    dist = np.sqrt((dv ** 2).sum(-1, keepdims=True))  # [B, E, 1]
    return dv.astype(np.float32), dist[..., 0].astype(np.float32)


def kernel(soft_assign, xyz, cg_xyz, assign_norm, assign_idx,
           w0, b0, w1, b1, w2, b2):
    global LAST_RESULT
    # If BASS_TRACE is set in an environment whose antenv lacks axon_hooks,
    # concourse's trace path would crash on import; register a stub registry
    # so tracing degrades gracefully instead.
    try:
        import antenv.axon_hooks  # noqa: F401
    except ImportError:
        import sys, types
        _m = types.ModuleType("antenv.axon_hooks")
        _m._hook = None
        _m.set_axon_ntff_profile_hook = lambda h: setattr(_m, "_hook", h)
        _m.get_axon_ntff_profile_hook = lambda: _m._hook
        sys.modules["antenv.axon_hooks"] = _m
    from concourse.bass_utils import run_bass_kernel_spmd

    soft_assign = np.asarray(soft_assign)
    xyz = np.asarray(xyz)
    cg_xyz = np.asarray(cg_xyz, dtype=np.float32)
    assign_norm = np.asarray(assign_norm, dtype=np.float32)
    idx = np.asarray(assign_idx).astype(np.int64)
    w0 = np.asarray(w0, dtype=np.float32); b0 = np.asarray(b0, dtype=np.float32)
    w1 = np.asarray(w1, dtype=np.float32); b1 = np.asarray(b1, dtype=np.float32)
    w2 = np.asarray(w2, dtype=np.float32); b2 = np.asarray(b2, dtype=np.float32)

    dv, dist = _host_prep(cg_xyz)                 # [B,E,3], [B,E]

    # shared (weight) inputs
    w0c = np.ascontiguousarray(w0.reshape(KT, 128).T, dtype=np.float32)
    b0c = np.ascontiguousarray(b0.reshape(KT, 128).T, dtype=np.float32)
    w1t = np.ascontiguousarray(
        w1.reshape(KT, 128, W).transpose(1, 0, 2).reshape(128, KT * W)).astype(_BF)
    w2t = np.ascontiguousarray(
        w2.reshape(KT, 128, W).transpose(1, 0, 2).reshape(128, KT * W)).astype(_BF)
    b1r = b1.reshape(1, W).astype(_BF)
    b2r = b2.reshape(1, W).astype(_BF)
    ones = np.ones((1, 128), dtype=_BF)
    id3 = np.eye(3, dtype=np.float32)

    in_maps = []
    for c in range(N_CORES):
        bs = slice(BPC * c, BPC * (c + 1))
        dvc = dv[bs].reshape(E2, 3)               # [2048, 3]
        distc = dist[bs].reshape(E2)
        dist_bf = distc.astype(_BF)
        dist_bc = np.ascontiguousarray(np.broadcast_to(dist_bf[None, :], (128, E2)))
        dvt = np.ascontiguousarray(
            dvc.reshape(ET, 128, 3).transpose(1, 0, 2).reshape(128, ET * 3)).astype(_BF)
        s2 = dv[bs].sum(axis=1).reshape(1, NB).astype(_BF)   # [1, 6]
        anp = assign_norm[bs]                     # [2, 1024, 128]
        ant = np.ascontiguousarray(
            anp.reshape(BPC, KT, 128, N_CGS).transpose(2, 0, 1, 3)
            .reshape(128, BPC * KT * 128)).astype(_BF)
        in_maps.append({
            "dist_bc": dist_bc, "w0c": w0c, "b0c": b0c,
            "w1t": w1t, "w2t": w2t, "b1r": b1r, "b2r": b2r,
            "ones": ones, "s2": s2, "dvt": dvt, "ant": ant, "id3": id3,
        })

    nc = _get_nc()
    res = run_bass_kernel_spmd(nc, in_maps, list(range(N_CORES)), trace=TRACE)
    LAST_RESULT = res

    xyz_recon = np.empty((B, N_ATOMS, 3), dtype=np.float32)
    for c in range(N_CORES):
        dxo = res.results[c]["dxo"]               # [128, KT*6]
        cgo = res.results[c]["cgo"]               # [128, 6]
        dx = dxo.reshape(128, KT, BPC, 3).transpose(2, 1, 0, 3).reshape(BPC, W, 3)
        cg = cgo.reshape(128, BPC, 3).transpose(1, 0, 2)      # [2, 128, 3]
        for j in range(BPC):
            b = BPC * c + j
            xyz_recon[b] = cg_xyz[b][idx] - cg[j][idx] + dx[j]

    return (soft_assign, xyz, xyz_recon)


# revision 44
# speedup vs baseline: 1.0642x; 1.0215x over previous
"""Trainium2 Bass kernel for nn_EquiMLP (gnn_message_passing).

Reference computation per batch b (B=16, n_cgs=128, n_atoms=W=1024, knn=8,
E = n_cgs*knn = 1024 edges):
  d2     = pairwise sq dists of cg_xyz[b]           [128,128]
  knbrs  = argsort(d2)[:, 1:9]                      [128,8]
  dv     = nbr_xyz - cg_xyz  (flattened)            [E,3]
  dist   = |dv|                                     [E]
  h0 = relu(dist*w0 + b0); h = relu(h0@w1 + b1)     [E,W]
  coeffs = h@w2 + b2                                [E,W]
  dx     = coeffs^T @ dv                            [W,3]
  cg     = an^T @ dx        (an = assign_norm[b])   [128,3]
  out    = cg_xyz[idx] - cg[idx] + dx               [1024,3]

Key algebraic restructure: coeffs (E x n_atoms, 67MB across batches) is never
materialized.  Since dx = (h@w2 + b2)^T @ dv = w2^T @ (h^T@dv) + b2 (x) sum(dv),
we contract over edges FIRST: g = h^T @ dv  [W,3], then dx = w2^T@g + b2 (x) s.
This removes one of the two E*W*W matmuls entirely.

Sharding: data-parallel over batch; core c computes batches {2c, 2c+1} fused
(2048 edges).  knn/argsort preprocessing and final index-gather assembly run
on host (tiny); all matmuls/activations run on device in bf16 with fp32 psum
accumulation.

Device layout notes (TRN2 matmul: out[M,N] = lhsT[K,M].T @ rhs[K,N], K =
partition dim):
  h0T  [j1-tile 128, e 2048] generated on ScalarE: relu(dist_bc*w0 + b0)
  z    [e 128, j2 1024] psum  = sum_k h0T_k.T @ w1_k   (+ ones^T@b1 rank-1)
  h    [e 128, j2 1024] bf16 sbuf = relu(z)  (VectorE, psum->sbuf cast)
  gT_b [3, j 1024] psum += dv_tile.T @ h_tile          (contract over edges)
  g2   [j 128-tiles, 6] via PE transpose of gT
  dx   [o 128-tiles, 6] psum = sum_k w2_k.T @ g2_k + b2 (x) s2
  cg   [c 128, 6] psum = sum_i an_i.T @ dx_i

Hardware constraint honored throughout: a PE instruction (Matmult/Ldweights)
can carry at most ONE semaphore wait, so "observer" matmuls absorb DMA-queue
waits into psum columns that are subsequently overwritten (start=True), and
the gT matmuls are software-pipelined one chunk behind the z matmuls so psum
recycling needs no extra DVE wait on the z matmuls.
"""

import numpy as np
import ml_dtypes

B, N_CGS, N_ATOMS, KNN = 16, 128, 1024, 8
W = N_ATOMS
N_CORES = 8
BPC = B // N_CORES          # batches per core = 2
E = N_CGS * KNN             # edges per batch = 1024
E2 = BPC * E                # edges per core = 2048
ET = E2 // 128              # e-tiles per core = 16
KT = W // 128               # k-tiles = 8
NB = BPC * 3                # fused xyz columns = 6

_BF = ml_dtypes.bfloat16
_BUILT = {}
LAST_RESULT = None          # BassKernelResults of the last run (for test.py)
TRACE = False               # set True from test.py to profile


def _build():
    import concourse.bass as bass
    import concourse.mybir as mybir
    from concourse.tile import TileContext
    from concourse.vector_clock import ScopedClock

    # The kernel-tail drain gets one wait per live semaphore (~11), but every
    # TPB instruction has a single HW wait slot.  Pre-emit standalone sync
    # wait_ge instructions (one per sem) before the drain; the post-pass then
    # strips the drain down to its last wait.
    _orig_dab = TileContext._drain_and_barrier

    def _patched_dab(self, tick_clock, wait_clock):
        probe = self.nc.sync.nop(hint="drain_wait_probe").ins
        wait_clock.add_sem_waits(probe, ScopedClock({None: tick_clock.global_clock}))
        waits = list(probe.sync_info.on_wait) if probe.sync_info and probe.sync_info.on_wait else []
        if waits:
            probe.sync_info.on_wait = [waits[-1]]
        handles = {h.name: h for h in self.sems.allocated().values()}
        for w in waits:
            h = handles.get(str(w.ant_name))
            if h is not None:
                self.nc.sync.wait_ge(h, w.wait_value)
            else:
                raise RuntimeError(f"no sem handle for {w.ant_name}")
        _orig_dab(self, tick_clock, wait_clock)

    TileContext._drain_and_barrier = _patched_dab

    bf = mybir.dt.bfloat16
    f32 = mybir.dt.float32
    nc = bass.Bass()

    # ---- DRAM I/O ----
    P = lambda name, shape, dt_: nc.declare_dram_parameter(name, shape, dt_, isOutput=False)
    dist_bc_d = P("dist_bc", [128, E2], bf)          # dist broadcast over partitions
    w0c_d = P("w0c", [128, KT], f32)                 # per-partition scales
    b0c_d = P("b0c", [128, KT], f32)
    w1t_d = P("w1t", [128, KT * W], bf)              # k-tiled w1 (rows k*128..)
    w2t_d = P("w2t", [128, KT * W], bf)              # k-tiled w2
    b1r_d = P("b1r", [1, W], bf)
    b2r_d = P("b2r", [1, W], bf)
    ones_d = P("ones", [1, 128], bf)
    s2_d = P("s2", [1, NB], bf)                      # per-batch sum(dv)
    dvt_d = P("dvt", [128, ET * 3], bf)              # e-tiled dist_vec
    ant_d = P("ant", [128, BPC * KT * 128], bf)      # i-tiled assign_norm
    id3_d = P("id3", [3, 3], f32)
    dxo_d = nc.declare_dram_parameter("dxo", [128, KT * NB], f32, isOutput=True)
    cgo_d = nc.declare_dram_parameter("cgo", [128, NB], f32, isOutput=True)
    scr1_d = nc.dram_tensor("scr1", [1, 8], f32)
    scr2_d = nc.dram_tensor("scr2", [1, 8], f32)

    with TileContext(nc) as tc:
        from contextlib import ExitStack
        with ExitStack() as ctx:
            csb = ctx.enter_context(tc.tile_pool(name="consts", bufs=1))
            wsb = ctx.enter_context(tc.tile_pool(name="weights", bufs=1))
            hsb = ctx.enter_context(tc.tile_pool(name="h", bufs=3))
            osb = ctx.enter_context(tc.tile_pool(name="outs", bufs=1))
            ps_g = ctx.enter_context(tc.tile_pool(name="ps_g", bufs=1, space="PSUM"))

            # ---- SBUF tiles + loads ----
            dist_bc = csb.tile([128, E2], bf)
            w0c = csb.tile([128, KT], f32)
            b0c = csb.tile([128, KT], f32)
            b1r = csb.tile([1, W], bf)
            b2r = csb.tile([1, W], bf)
            ones = csb.tile([1, 128], bf)
            s2 = csb.tile([1, NB], bf)
            dvt = csb.tile([128, ET * 3], bf)
            ant = csb.tile([128, BPC * KT * 128], bf)
            id3 = csb.tile([3, 3], f32)
            w1t = wsb.tile([128, KT * W], bf)
            w2t = wsb.tile([128, KT * W], bf)
            h0T = wsb.tile([128, KT * E2], bf)

            sd = nc.sync.dma_start
            # Startup-critical DMAs go on the GpSimd queue (its sequencer
            # exits the preamble earliest) with warmup inputs first, so the
            # HAM warmup matmuls can start as soon as PE's preamble ends.
            gd = nc.gpsimd.dma_start
            gd(out=w0c[:], in_=w0c_d[:])
            gd(out=b0c[:], in_=b0c_d[:])
            gd(out=dist_bc[:, 0:256], in_=dist_bc_d[:, 0:256])
            gd(out=ones[:], in_=ones_d[:])
            gd(out=b1r[:], in_=b1r_d[:])
            gd(out=dist_bc[:, 256:E2 // 2], in_=dist_bc_d[:, 256:E2 // 2])
            gd(out=dist_bc[:, E2 // 2:], in_=dist_bc_d[:, E2 // 2:])
            sd(out=b2r[:], in_=b2r_d[:])
            sd(out=s2[:], in_=s2_d[:])
            sd(out=id3[:], in_=id3_d[:])
            sd(out=dvt[:], in_=dvt_d[:])
            for k in range(KT):
                sd(out=w1t[:, k * W:(k + 1) * W], in_=w1t_d[:, k * W:(k + 1) * W])
            sd(out=ant[:], in_=ant_d[:])
            for k in range(KT):
                sd(out=w2t[:, k * W:(k + 1) * W], in_=w2t_d[:, k * W:(k + 1) * W])

            # gT accumulators live across the whole main loop.  Packed as
            # [35, 512] (row 0-2 = j 0:512, rows 32-34 = j 512:1024) so each
            # fits ONE psum bank, freeing a bank for the HAM warmup tile.
            gT = [ps_g.tile([35, 512], f32, tag=f"gT{b}", name=f"gT{b}")
                  for b in range(BPC)]

            Relu = mybir.ActivationFunctionType.Relu

            with tc.tile_pool(name="ps_z", bufs=2, space="PSUM") as ps_z:
                # ---- observers: absorb DMA-queue waits into PE, one per inst.
                # They write z-psum columns that k==0/start=True later clears.
                zobs = ps_z.tile([128, 1024], f32, tag="z")
                # HAM warmup FIRST in the PE stream (only needs ones/b1r,
                # which are DMA'd early): dense real matmuls into a dedicated
                # psum bank nothing else touches, keeping the PE activity
                # monitor hot through the phase-A/DMA startup window.
                wup = ps_g.tile([128, 512], f32, tag="wup", name="wup")
                NWUP = 12
                for i in range(NWUP):
                    nc.tensor.matmul(wup[:, :], ones[:, :], b1r[:, 0:512],
                                     start=(i == 0), stop=(i == NWUP - 1))
                obs_targets = [s2, dvt, id3] + \
                    [w1t[:, k * W: k * W + 128] for k in range(KT)]
                # out [1,1] = t[:, :1].T @ t[:, :1] — absorbs one DMA wait each
                for t in obs_targets:
                    nc.tensor.matmul(zobs[0:1, 0:1], t[:, 0:1], t[:, 0:1],
                                     start=True, stop=True)

                # ---- ACT observers: absorb phase-A input DMA waits, one per op
                sscr = csb.tile([1, 8], f32, name="sscr")
                act_obs = [nc.scalar.copy(sscr[0:1, i:i + 1], t)
                           for i, t in enumerate(
                               (dist_bc[0:1, 0:1],
                                dist_bc[0:1, 256:257],
                                dist_bc[0:1, E2 // 2:E2 // 2 + 1],
                                w0c[0:1, 0:1], b0c[0:1, 0:1]))]

                # ---- phase A: h0T generation, e-superchunk-major so the z loop
                # can start after the first superchunk.
                sc_edges = [0, 256, 512, 1024, 1536, 2048]
                from concourse.bass import _add_dep_helper
                for sc in range(len(sc_edges) - 1):
                    lo, hi = sc_edges[sc], sc_edges[sc + 1]
                    for m in range(KT):
                        a = nc.scalar.activation(
                            h0T[:, m * E2 + lo: m * E2 + hi],
                            dist_bc[:, lo:hi],
                            Relu,
                            bias=b0c[:, m:m + 1],
                            scale=w0c[:, m:m + 1],
                        )
                        if sc == 0:
                            # force the observers to schedule before phase A
                            for o in act_obs:
                                _add_dep_helper(a.ins, o.ins, sync=False,
                                                reason="act-obs order")

                # ---- main loop: z -> relu -> (pipelined) gT
                h_tiles = [None] * ET

                gT_last_by_ec = {}

                def emit_z(ec):
                    z = ps_z.tile([128, 1024], f32, tag="z", name="z")
                    for n in range(2):
                        # rank-1 bias row opens the accumulation region
                        # (start=True, no h0T dep, absorbs psum epoch waits)
                        nc.tensor.matmul(
                            z[:, n * 512:(n + 1) * 512],
                            ones[:, :],
                            b1r[:, n * 512:(n + 1) * 512],
                            start=True, stop=False)
                        for k in range(KT):
                            nc.tensor.matmul(
                                z[:, n * 512:(n + 1) * 512],
                                h0T[:, k * E2 + ec * 128: k * E2 + (ec + 1) * 128],
                                w1t[:, k * W + n * 512: k * W + (n + 1) * 512],
                                start=False, stop=(k == KT - 1))
                    h = hsb.tile([128, W], bf, tag="h")
                    nc.vector.tensor_scalar_max(h[:], z[:], 0.0)
                    h_tiles[ec] = h

                last_gT = [None]

                def emit_gT(ec):
                    b = ec // KT
                    lec = ec % KT
                    for n in range(2):
                        last_gT[0] = nc.tensor.matmul(
                            gT[b][32 * n:32 * n + 3, :],
                            dvt[:, ec * 3:(ec + 1) * 3],
                            h_tiles[ec][:, n * 512:(n + 1) * 512],
                            start=(lec == 0), stop=(lec == KT - 1))
                    gT_last_by_ec[ec] = last_gT[0]

                for ec in range(ET):
                    emit_z(ec)
                    if ec >= 1:
                        emit_gT(ec - 1)
                emit_gT(ET - 1)

            # ---- tail ----
            with tc.tile_pool(name="ps_t", bufs=1, space="PSUM") as ps_t:
                gT_sb = [osb.tile([3, W], f32, tag=f"gTs{b}", name=f"gTs{b}")
                         for b in range(BPC)]
                for b in range(BPC):
                    nc.scalar.copy(gT_sb[b][:, 0:512], gT[b][0:3, :])
                    nc.scalar.copy(gT_sb[b][:, 512:1024], gT[b][32:35, :])

                # transpose gT -> g2 [128, KT*6], per batch (partition base 0)
                g2p = ps_t.tile([128, KT * NB], f32, tag="g2p")
                # PE observer for the ps_t pool-overlap waits; forced after
                # the final gT matmul so its DVE released-zone component is
                # already observed by the PE clock (single wait left).
                g2p_obs = nc.tensor.matmul(g2p[0:1, 0:1], id3[:, 0:1],
                                           id3[:, 0:1], start=True, stop=True)
                _add_dep_helper(g2p_obs.ins, last_gT[0].ins, sync=False,
                                reason="tail-obs after last gT")
                for t in range(KT):
                    for b in range(BPC):
                        nc.tensor.transpose(
                            g2p[:, t * NB + 3 * b: t * NB + 3 * b + 3],
                            gT_sb[b][:, t * 128:(t + 1) * 128],
                            id3[:, :])
                g2 = osb.tile([128, KT * NB], bf)
                vscr = csb.tile([1, 8], f32, name="vscr")
                # DVE observer: absorb the PE (transposes-done) wait.  The g2
                # copy runs only after ALL transposes: a slice-wise pipeline
                # here races PE-writes vs DVE-reads on the single g2p psum
                # bank (hard fault / corruption).
                nc.vector.tensor_copy(vscr[0:1, 0:1], g2p[0:1, 0:1])
                nc.vector.tensor_copy(g2[:], g2p[:])

                # dx = sum_k w2_k.T @ g2_k + b2 (x) s2
                dxp = ps_t.tile([128, KT * NB], f32, tag="dxp")
                # observers for w2t chunks (+ g2 DVE) land in dxp col 0
                for k in range(KT):
                    o = nc.tensor.matmul(dxp[0:1, 0:1],
                                         w2t[:, k * W: k * W + 1],
                                         w2t[:, k * W: k * W + 1],
                                         start=True, stop=True)
                    if k == 0:
                        _add_dep_helper(o.ins, last_gT[0].ins, sync=False,
                                        reason="tail-obs after last gT")
                nc.tensor.matmul(dxp[0:1, 0:1], g2[:, 0:1], g2[:, 0:1],
                                 start=True, stop=True)
                for oc in range(KT):
                    nc.tensor.matmul(
                        dxp[:, oc * NB:(oc + 1) * NB],
                        b2r[:, oc * 128:(oc + 1) * 128],
                        s2[:, :],
                        start=True, stop=False)
                    for k in range(KT):
                        nc.tensor.matmul(
                            dxp[:, oc * NB:(oc + 1) * NB],
                            w2t[:, k * W + oc * 128: k * W + (oc + 1) * 128],
                            g2[:, k * NB:(k + 1) * NB],
                            start=False, stop=(k == KT - 1))

                dxo = osb.tile([128, KT * NB], f32)
                dxb = osb.tile([128, KT * NB], bf)
                # observer: DVE absorbs the PE (dx-done) wait first
                nc.vector.tensor_copy(vscr[0:1, 1:2], dxp[0:1, 0:1])
                nc.vector.tensor_copy(dxo[:], dxp[:])
                nc.vector.tensor_copy(dxb[:], dxp[:])
                # dummy 4B DMA absorbs the DVE wait; the real out-DMA then
                # carries only its DRAM-page WAR wait (1 slot each).
                nc.sync.dma_start(out=scr1_d[0:1, 0:1], in_=dxo[0:1, 0:1])
                nc.sync.dma_start(out=dxo_d[:], in_=dxo[:])

                # cg = sum_i an_i.T @ dx_i   per batch
                cgp = ps_t.tile([128, NB], f32, tag="cgp")
                o = nc.tensor.matmul(cgp[0:1, 0:1], ant[:, 0:1], ant[:, 0:1],
                                     start=True, stop=True)
                _add_dep_helper(o.ins, last_gT[0].ins, sync=False,
                                reason="tail-obs after last gT")
                nc.tensor.matmul(cgp[0:1, 0:1], dxb[:, 0:1], dxb[:, 0:1],
                                 start=True, stop=True)
                for b in range(BPC):
                    for it in range(KT):
                        nc.tensor.matmul(
                            cgp[:, 3 * b:3 * (b + 1)],
                            ant[:, (b * KT + it) * 128:(b * KT + it + 1) * 128],
                            dxb[:, it * NB + 3 * b: it * NB + 3 * b + 3],
                            start=(it == 0), stop=(it == KT - 1))
                cgo = osb.tile([128, NB], f32)
                nc.vector.tensor_copy(cgo[:], cgp[:])
                nc.sync.dma_start(out=scr2_d[0:1, 0:1], in_=cgo[0:1, 0:1])
                nc.sync.dma_start(out=cgo_d[:], in_=cgo[:])

    TileContext._drain_and_barrier = _orig_dab
    _strip_self_waits(nc)
    return nc


# Engine instruction families -> the engine's own completion-sem prefix.
# An instruction waiting on its OWN engine's sem is trivially satisfied at
# runtime for the strictly in-order ACT/DVE queues (RAR/epoch bookkeeping the
# scheduler fails to elide), but it consumes the single HW wait slot.  PE
# self-waits are NOT stripped (psum fill/drain overlap makes them real).
_SELF_SEM = {
    "InstTensorScalarPtr": "DVE_",
    "InstTensorCopy": "DVE_",
    "InstTensorTensor": "DVE_",
    "InstMemset": "DVE_",
    "InstActivation": "Activation_",
}


def _strip_self_waits(nc):
    for bb in nc.m.functions[0].blocks:
        for inst in bb.instructions:
            si = getattr(inst, "sync_info", None)
            if not si or not si.on_wait or len(si.on_wait) < 2:
                continue
            tn = type(inst).__name__
            if tn == "InstDrain":
                # covered by the pre-emitted wait_ge chain (_patched_dab)
                si.on_wait = [si.on_wait[-1]]
                continue
            if tn == "InstDMACopy":
                # A DMA waiting on the sem of its OWN hardware queue is
                # redundant: per-queue descriptor execution is FIFO.
                own = {str(u.ant_name) for u in (si.on_update or [])}
                kept = [w for w in si.on_wait if str(w.ant_name) not in own]
                if len(kept) != len(si.on_wait) and kept:
                    si.on_wait = kept
                continue
            pref = _SELF_SEM.get(tn)
            if pref is None:
                continue
            kept = [w for w in si.on_wait if not str(w.ant_name).startswith(pref)]
            if len(kept) != len(si.on_wait) and kept:
                si.on_wait = kept


def _enable_ldw_opt():  # unused: walrus rejects pre-split InstLdweights
    # The toolchain invokes walrus with --enable-ldw-opt=false, which keeps
    # every MATMUL serialized behind its LDWEIGHTS (~380ns instead of ~215ns
    # per N=512 bf16 matmul).  Enabling it fails on Tile's pre-split
    # InstLdweights ("not compatible with LDW optimization"), and the split
    # happens in compiled tile_legalize.rs — not reachable from here.
    import concourse.bass_utils as bu
    if getattr(bu.run_command, "_ldw_patched", False):
        return
    orig = bu.run_command

    def patched(cmd, **kw):
        cmd = [("--enable-ldw-opt=true" if c == "--enable-ldw-opt=false" else c)
               for c in cmd]
        return orig(cmd, **kw)

    patched._ldw_patched = True
    bu.run_command = patched


def _get_nc():
    if "nc" not in _BUILT:
        _BUILT["nc"] = _build()
    return _BUILT["nc"]


def _host_prep(cg_xyz):
    """Exact replication of the reference knn/edge construction (fp32)."""
    diff = cg_xyz[:, :, None, :] - cg_xyz[:, None, :, :]
    d2 = (diff ** 2).sum(-1)                      # [B, 128, 128] fp32
    knbrs = np.argsort(d2, axis=-1, kind="stable")[:, :, 1:KNN + 1]
    nbr = np.stack([cg_xyz[b][knbrs[b]] for b in range(cg_xyz.shape[0])])
    dv = (nbr - cg_xyz[:, :, None, :]).reshape(cg_xyz.shape[0], E, 3)
    dist = np.sqrt((dv ** 2).sum(-1, keepdims=True))  # [B, E, 1]
    return dv.astype(np.float32), dist[..., 0].astype(np.float32)


def kernel(soft_assign, xyz, cg_xyz, assign_norm, assign_idx,
           w0, b0, w1, b1, w2, b2):
    global LAST_RESULT
    # If BASS_TRACE is set in an environment whose antenv lacks axon_hooks,
    # concourse's trace path would crash on import; register a stub registry
    # so tracing degrades gracefully instead.
    try:
        import antenv.axon_hooks  # noqa: F401
    except ImportError:
        import sys, types
        _m = types.ModuleType("antenv.axon_hooks")
        _m._hook = None
        _m.set_axon_ntff_profile_hook = lambda h: setattr(_m, "_hook", h)
        _m.get_axon_ntff_profile_hook = lambda: _m._hook
        sys.modules["antenv.axon_hooks"] = _m
    from concourse.bass_utils import run_bass_kernel_spmd

    soft_assign = np.asarray(soft_assign)
    xyz = np.asarray(xyz)
    cg_xyz = np.asarray(cg_xyz, dtype=np.float32)
    assign_norm = np.asarray(assign_norm, dtype=np.float32)
    idx = np.asarray(assign_idx).astype(np.int64)
    w0 = np.asarray(w0, dtype=np.float32); b0 = np.asarray(b0, dtype=np.float32)
    w1 = np.asarray(w1, dtype=np.float32); b1 = np.asarray(b1, dtype=np.float32)
    w2 = np.asarray(w2, dtype=np.float32); b2 = np.asarray(b2, dtype=np.float32)

    dv, dist = _host_prep(cg_xyz)                 # [B,E,3], [B,E]

    # shared (weight) inputs
    w0c = np.ascontiguousarray(w0.reshape(KT, 128).T, dtype=np.float32)
    b0c = np.ascontiguousarray(b0.reshape(KT, 128).T, dtype=np.float32)
    w1t = np.ascontiguousarray(
        w1.reshape(KT, 128, W).transpose(1, 0, 2).reshape(128, KT * W)).astype(_BF)
    w2t = np.ascontiguousarray(
        w2.reshape(KT, 128, W).transpose(1, 0, 2).reshape(128, KT * W)).astype(_BF)
    b1r = b1.reshape(1, W).astype(_BF)
    b2r = b2.reshape(1, W).astype(_BF)
    ones = np.ones((1, 128), dtype=_BF)
    id3 = np.eye(3, dtype=np.float32)

    in_maps = []
    for c in range(N_CORES):
        bs = slice(BPC * c, BPC * (c + 1))
        dvc = dv[bs].reshape(E2, 3)               # [2048, 3]
        distc = dist[bs].reshape(E2)
        dist_bf = distc.astype(_BF)
        dist_bc = np.ascontiguousarray(np.broadcast_to(dist_bf[None, :], (128, E2)))
        dvt = np.ascontiguousarray(
            dvc.reshape(ET, 128, 3).transpose(1, 0, 2).reshape(128, ET * 3)).astype(_BF)
        s2 = dv[bs].sum(axis=1).reshape(1, NB).astype(_BF)   # [1, 6]
        anp = assign_norm[bs]                     # [2, 1024, 128]
        ant = np.ascontiguousarray(
            anp.reshape(BPC, KT, 128, N_CGS).transpose(2, 0, 1, 3)
            .reshape(128, BPC * KT * 128)).astype(_BF)
        in_maps.append({
            "dist_bc": dist_bc, "w0c": w0c, "b0c": b0c,
            "w1t": w1t, "w2t": w2t, "b1r": b1r, "b2r": b2r,
            "ones": ones, "s2": s2, "dvt": dvt, "ant": ant, "id3": id3,
        })

    nc = _get_nc()
    res = run_bass_kernel_spmd(nc, in_maps, list(range(N_CORES)), trace=TRACE)
    LAST_RESULT = res

    xyz_recon = np.empty((B, N_ATOMS, 3), dtype=np.float32)
    for c in range(N_CORES):
        dxo = res.results[c]["dxo"]               # [128, KT*6]
        cgo = res.results[c]["cgo"]               # [128, 6]
        dx = dxo.reshape(128, KT, BPC, 3).transpose(2, 1, 0, 3).reshape(BPC, W, 3)
        cg = cgo.reshape(128, BPC, 3).transpose(1, 0, 2)      # [2, 128, 3]
        for j in range(BPC):
            b = BPC * c + j
            xyz_recon[b] = cg_xyz[b][idx] - cg[j][idx] + dx[j]

    return (soft_assign, xyz, xyz_recon)
